# revision 36
# baseline (speedup 1.0000x reference)
"""Trainium2 Bass kernel for nn_DeforConv_71605694759687.

ResBlock(stride2, 64->128) + DCNv2 (modulated deformable conv) + BN + ReLU.

Sharding (8 cores): (batch b = core//4, H-quarter q = core%4); each core
computes 32 output rows of out[b] end-to-end locally (halo via recompute,
no collectives).

DCNv2 via TRUE bilinear gather: Pool's ap_gather fetches (x0, x0+1)
feature pairs (as I32 on an x-duplicated fp16 field) at per-pixel corner
positions for both y corners of each of the 9 taps.  Corner weights
mask*(1-fy)(1-fx) etc. are computed per (tap, group) on 18 partitions,
exported to a DRAM scratch in gather-matched (yc,qq,r,c,xc) order, and
partition-broadcast to the 128 channel partitions by a single stride-0
DMA per tap (128 descriptors x 16KB) -- no PE/Act involvement.  One
fp16 TensorTensor (2x DVE mode) per tap Hadamard-multiplies weights
into the gathered pairs; 16 accumulating PE matmuls per tap contract
(group,channel) with the DCN weight into the output PSUM.

The whole kernel is one software pipeline: conv work that pass-0's DCN
doesn't need (conv1 7-9, conv2 6-9, offset 4-7) plus pass-1's index /
corner-weight prep are deferred into pass-0's DMA-bound k-loop gaps, so
the PE never idles long enough to drop out of max p-state and pass-1's
gather pipeline starts without a bubble.

Gather pixel order (per 2048-px pass): col i = yc*2048 + qq*1024 +
r*128 + c, so idx i sits at partition c%16 of each 16-partition gpsimd
group; the (r,cH,cL)->(cL,r,cH) index shuffle is absorbed into the
idx16 write AP (free), and all exports/wraps lower to fat strided DMAs.
"""

import numpy as np
import ml_dtypes
from contextlib import ExitStack

import concourse.bass as bass
import concourse.tile as tile
from concourse import mybir, bacc
from concourse.bass_utils import run_bass_kernel_spmd

F32 = mybir.dt.float32
F16 = mybir.dt.float16
BF16 = mybir.dt.bfloat16
I16 = mybir.dt.int16
I32 = mybir.dt.int32
AL = mybir.AluOpType
AF = mybir.ActivationFunctionType

P = 128
EPS = 1e-5
Ci, Co, DG, Cg = 64, 128, 2, 64
H, W = 128, 128          # output spatial (after stride-2)
QROWS = 32               # output rows per core
FR, FC = 38, 134         # F field: rows h0-3..h0+34, cols x in [-3,130]
F1R, F1C = 40, 130       # feat1: rows h0-4..h0+35, cols [-1,128]
XR, XC = 81, 258         # x_pad: rows 2*h0-9..2*h0+71, cols [-1,256]
NCHUNK = 1024
FLAT = FR * FC           # 5092
INW = 21 * FC + 132 + 1  # 2947: gather in-window elems per pass


def _h(x):
    return np.ascontiguousarray(np.asarray(x, dtype=np.float32).astype(np.float16))


def _f(x):
    return np.ascontiguousarray(np.asarray(x, dtype=np.float32))


def build_nc():
    nc = bacc.Bacc(None)

    d_x = nc.dram_tensor("x_shard", [Ci, XR, XC], F16, kind="ExternalInput")
    d_l1 = nc.dram_tensor("lhsT1", [Ci, 9, P], F16, kind="ExternalInput")
    d_l2 = nc.dram_tensor("lhsT2", [P, 9, P], F16, kind="ExternalInput")
    d_lsc = nc.dram_tensor("lhsT_sc", [Ci, P], F16, kind="ExternalInput")
    d_loff = nc.dram_tensor("lhsT_off", [P, 9, 54], F16, kind="ExternalInput")
    d_ldcn = nc.dram_tensor("lhsT_dcn", [P, 9, P], F16, kind="ExternalInput")
    d_cst = nc.dram_tensor("consts", [P, 8], F32, kind="ExternalInput")
    d_bq = nc.dram_tensor("bias_q", [P, 3], F32, kind="ExternalInput")
    d_by = nc.dram_tensor("base_y", [P, NCHUNK], F16, kind="ExternalInput")
    d_bx = nc.dram_tensor("base_x", [P, NCHUNK], F16, kind="ExternalInput")
    d_rm1 = nc.dram_tensor("rowmask1", [P, F1R], F16, kind="ExternalInput")
    d_rmf = nc.dram_tensor("rowmaskF", [P, FR], F16, kind="ExternalInput")
    # out layout: (pass, qq, r, c) rows 16*pp+8*qq+r
    d_out = nc.dram_tensor("out", [P, 2, 2, NCHUNK], F16, kind="ExternalOutput")
    # idx scratch [pass][t=2k+dd][cL][qq][r][cH]: wrap per (pass,dd) is one
    # broadcast DMA with 256B runs
    d_scr = nc.dram_tensor("idx_scratch", [2, 18, 16, 2, 8, 8], I16,
                           kind="Internal")
    # corner-weight scratch [pass][t][yc][qq][r][c][xc]: per-k broadcast
    # reads [2, 8192] contiguous rows
    d_cw = nc.dram_tensor("cw_scratch", [2, 18, 2, 2, 8, 128, 2], F16,
                          kind="Internal")

    with tile.TileContext(nc) as tc, ExitStack() as ctx:
        singles = ctx.enter_context(tc.tile_pool(name="singles", bufs=1))

        # ---- persistent SBUF ----
        # F field split per pass (x-pair duplicated): fd_a rows 0..23,
        # fd_b rows 16..37 -- so pass-0 gathers don't wait on late conv2
        fd_a = singles.tile([P, 24 * FC, 2], F16)
        fd_b = singles.tile([P, 22 * FC, 2], F16)
        ldcn = singles.tile([P, 9, P], F16)
        cst = singles.tile([P, 8], F32)
        cw = singles.tile([P, 2, NCHUNK, 2], F16)   # corner wts (yc,px,xc)
        # wrapped gather idxs, one tile per pass: [p, k, (y0|y1 idx)]
        idxw_t = [singles.tile([P, 9, 256], I16, name=f"idxw{pp}")
                  for pp in range(2)]

        nc.sync.dma_start(out=ldcn[:], in_=d_ldcn[:])
        nc.sync.dma_start(out=cst[:], in_=d_cst[:])

        inv1, beta1 = cst[:, 0:1], cst[:, 1:2]
        inv2, beta2 = cst[:, 2:3], cst[:, 3:4]
        inv3, beta3 = cst[:, 4:5], cst[:, 5:6]

        fda4 = fd_a[:].rearrange("p (r c) e -> p r c e", r=24)
        fdb4 = fd_b[:].rearrange("p (r c) e -> p r c e", r=22)
        # zero the x-halo columns (cols 0..2 and 131..133 of every row)
        nc.vector.memset(fda4[:, :, 0:3, :], 0.0)
        nc.vector.memset(fda4[:, :, FC - 3:FC, :], 0.0)
        nc.vector.memset(fdb4[:, :, 0:3, :], 0.0)
        nc.vector.memset(fdb4[:, :, FC - 3:FC, :], 0.0)

        ph_b = ctx.enter_context(tc.tile_pool(name="ph_b", bufs=1))
        pbt = ctx.enter_context(tc.tile_pool(name="ph_b_tmp", bufs=1))
        pa = ctx.enter_context(tc.tile_pool(name="ph_a", bufs=1))
        psa = ctx.enter_context(tc.tile_pool(name="psum_a", bufs=2,
                                             space="PSUM"))
        psb = ctx.enter_context(tc.tile_pool(name="psum_b", bufs=2,
                                             space="PSUM"))
        psum_out = ctx.enter_context(tc.tile_pool(name="psum_out", bufs=1,
                                                  space="PSUM"))
        gb_pool = ctx.enter_context(tc.tile_pool(name="gb", bufs=2))
        wcb_pool = ctx.enter_context(tc.tile_pool(name="wcb", bufs=4))
        rt_pool = ctx.enter_context(tc.tile_pool(name="rts", bufs=2))
        out_pool = ctx.enter_context(tc.tile_pool(name="outs", bufs=2))

        loff = ph_b.tile([P, 9, 54], F16)
        bq = ph_b.tile([P, 3], F32)
        basey = ph_b.tile([P, NCHUNK], F16)
        basex = ph_b.tile([P, NCHUNK], F16)
        q_t = ph_b.tile([P, 3, NCHUNK], F32)     # dy, dx, mm
        y0i = ph_b.tile([P, NCHUNK], I16)
        x0i = ph_b.tile([P, NCHUNK], I16)
        # scratch aliases: dy/dx planes of q_t are dead once yf/xf exist,
        # yf is dead once fy exists, y0i once y0f exists, and the mask /
        # fy tiles hold u0 / u1 after the in-place corner-weight products
        yf = q_t[:, 0, :]        # in-place: dy plane += base
        xf = q_t[:, 1, :]
        y0f = basey[:]           # floors are small exact ints in f16
        x0f = basex[:]
        idxf = q_t[:, 1, :]      # xf is dead once fx exists
        idx16 = y0i
        fy = x0i[:].bitcast(F16)   # x0i is dead once x0f exists
        # mm plane after sigmoid: m_t in the lower half (in-place-safe:
        # the f16 write stream never overtakes the f32 read), fx upper
        m_t = q_t[:, 2, :].bitcast(F16)[:, 0:NCHUNK]
        fx = q_t[:, 2, :].bitcast(F16)[:, NCHUNK:2 * NCHUNK]

        x_pad = pa.tile([Ci, XR, XC], F16)
        feat1 = pa.tile([P, F1R, F1C], F16)
        l1 = pa.tile([Ci, 9, P], F16)
        l2 = pa.tile([P, 9, P], F16)
        lsc = pa.tile([Ci, P], F16)
        rm1 = pa.tile([P, F1R], F16)
        rmf = pa.tile([P, FR], F16)

        nc.sync.dma_start(out=l1[:], in_=d_l1[:])
        for i in range(4):
            nc.sync.dma_start(out=x_pad[:, 3 * i: 3 * i + 3, :],
                              in_=d_x[:, 3 * i: 3 * i + 3, :])
        nc.sync.dma_start(out=x_pad[:, 12:XR, :], in_=d_x[:, 12:XR, :])
        for t, dref in ((l2, d_l2), (lsc, d_lsc),
                        (rm1, d_rm1), (rmf, d_rmf), (loff, d_loff),
                        (bq, d_bq), (basey, d_by), (basex, d_bx)):
            nc.sync.dma_start(out=t[:], in_=dref[:])

        nc.vector.memset(feat1[:, :, 0:1], 0.0)
        nc.vector.memset(feat1[:, :, F1C - 1:F1C], 0.0)

        # ---------------- chunk emitters ----------------
        def conv1_chunk(cki):
            # feat1 row f1 <-> global h0-4+f1; x_pad rows 2*f1+ty
            r0 = cki * 4
            ps = psa.tile([P, 4, W], F32, tag="ps12")
            for t in range(9):
                ty, tx = t // 3, t % 3
                rhs = x_pad[:, 2 * r0 + ty: 2 * r0 + ty + 7: 2,
                            tx: tx + 2 * W - 1: 2]
                nc.tensor.matmul(ps[:], l1[:, t, :], rhs,
                                 start=(t == 0), stop=(t == 8))
            nc.scalar.activation(feat1[:, r0:r0 + 4, 1:1 + W], ps[:],
                                 AF.Relu, bias=beta1, scale=inv1)
            # out-of-image rows only ever appear in chunks 0 and 9
            if cki in (0, 9):
                nc.vector.tensor_tensor(
                    feat1[:, r0:r0 + 4, :], feat1[:, r0:r0 + 4, :],
                    rm1[:, r0:r0 + 4, None].to_broadcast([P, 4, F1C]),
                    AL.mult)

        def conv2_chunk(cki):
            # F row f2 <-> global h0-3+f2; feat1 rows f2+ty
            r0 = cki * 4
            nrow = min(4, FR - r0)
            if cki <= 5:
                fdt, fdt4, rb = fd_a, fda4, r0
            else:
                fdt, fdt4, rb = fd_b, fdb4, r0 - 16
            ps = psa.tile([P, 4, W], F32, tag="ps12")
            for t in range(9):
                ty, tx = t // 3, t % 3
                rhs = feat1[:, r0 + ty: r0 + ty + nrow, tx: tx + W]
                nc.tensor.matmul(ps[:, :nrow], l2[:, t, :], rhs,
                                 start=(t == 0), stop=False)
            rhs_sc = x_pad[:, 2 * r0 + 3: 2 * r0 + 2 + 2 * nrow: 2,
                           1: 2 * W: 2]
            nc.tensor.matmul(ps[:, :nrow], lsc[:], rhs_sc,
                             start=False, stop=True)
            nc.scalar.activation(fdt4[:, rb:rb + nrow, 3:3 + W, 0],
                                 ps[:, :nrow], AF.Relu,
                                 bias=beta2, scale=inv2)
            if cki in (0, 8, 9):
                nc.vector.tensor_tensor(
                    fdt4[:, rb:rb + nrow, :, 0],
                    fdt4[:, rb:rb + nrow, :, 0],
                    rmf[:, r0:r0 + nrow, None].to_broadcast([P, nrow, FC]),
                    AL.mult)
            a0, a1 = rb * FC, (rb + nrow) * FC
            nc.scalar.copy(fdt[:, max(a0 - 1, 0):a1 - 1, 1],
                           fdt[:, max(a0, 1):a1, 0])
            if cki == 5:
                # seed fd_b rows 0..7 (global 16..23) from fd_a; chunk
                # 6's dup then fixes the row-7/col-133 pair boundary
                nc.scalar.copy(fd_b[:, 0:8 * FC, :],
                               fd_a[:, 16 * FC:24 * FC, :])
            if cki == 9:
                nc.vector.memset(fd_b[:, 22 * FC - 1: 22 * FC, 1], 0.0)

        def off_chunk(cki):
            # offset conv om: rows (k*2+d)*3+quant; out rows h0..h0+31
            r0 = cki * 4
            pq, c2 = cki // 2, cki % 2
            if cki <= 3:
                fdt4, rb = fda4, r0
            else:
                fdt4, rb = fdb4, r0 - 16
            ps = psb.tile([54, 4, W], F32)
            for t in range(9):
                ty, tx = t // 3, t % 3
                rhs = fdt4[:, rb + 2 + ty: rb + 6 + ty,
                           2 + tx: 2 + tx + W, 0]
                nc.tensor.matmul(ps[:], loff[:, t, :], rhs,
                                 start=(t == 0), stop=(t == 8))
            om_sb = pbt.tile([54, 512], F32, tag="om_sb")
            nc.scalar.copy(om_sb[:], ps[:].rearrange("p a b -> p (a b)"))
            for q in range(3):
                nc.sync.dma_start(
                    out=q_t[pq * 32: pq * 32 + 18, q,
                            c2 * 512:(c2 + 1) * 512],
                    in_=om_sb[q * 18:(q + 1) * 18, :])

        def phase_b_idx(pp):
            # index chain on partition half [64pp, 64pp+64)
            # (dy/dx offset-conv biases are folded into base_y/base_x
            # host-side)
            s = slice(64 * pp, 64 * pp + 64)
            nc.scalar.activation(m_t[s], q_t[s, 2, :], AF.Sigmoid,
                                 bias=bq[s, 2:3])
            nc.vector.tensor_tensor(yf[s], q_t[s, 0, :], basey[s], AL.add)
            nc.vector.tensor_tensor(xf[s], q_t[s, 1, :], basex[s], AL.add)
            # floor via RNE(v - 0.5), clamped to the legal corner range
            # floor-converts on Act (RNE f32->i16 is bit-identical there,
            # verified) to relieve the congested DVE
            nc.scalar.activation(y0i[s], yf[s], AF.Copy, bias=-0.5)
            nc.scalar.activation(x0i[s], xf[s], AF.Copy, bias=-0.5)
            nc.vector.tensor_scalar(y0i[s], y0i[s], 0, 20, AL.max, AL.min)
            nc.vector.tensor_scalar(x0i[s], x0i[s], 0, 132, AL.max, AL.min)
            nc.scalar.copy(y0f[s], y0i[s])
            nc.scalar.copy(x0f[s], x0i[s])
            # fractional parts, clamped to [0, 1] (before idxf clobbers yf)
            nc.vector.scalar_tensor_tensor(fy[s], y0f[s], -1.0, yf[s],
                                           AL.mult, AL.add)
            nc.vector.scalar_tensor_tensor(fx[s], x0f[s], -1.0, xf[s],
                                           AL.mult, AL.add)
            nc.vector.tensor_scalar(fy[s], fy[s], 0.0, 1.0, AL.max, AL.min)
            nc.vector.tensor_scalar(fx[s], fx[s], 0.0, 1.0, AL.max, AL.min)
            # flat gather index (pass-relative); the copy's out AP shuffles
            # (r, cH, cL) -> (cL, r, cH) so exports/wraps are fat DMAs
            nc.vector.scalar_tensor_tensor(idxf[s], y0f[s], float(FC),
                                           x0f[s], AL.mult, AL.add)
            # on Act: the values are exact small integers, so the f32->i16
            # convert is exact there too, and it leaves the congested DVE
            nc.scalar.copy(
                idx16[s].rearrange("p (cl r ch) -> p r ch cl",
                                   cl=16, r=8, ch=8),
                idxf[s].rearrange("p (r ch cl) -> p r ch cl",
                                  r=8, ch=8, cl=16))
            # export idx to DRAM: d_scr[pp][t][cL][qq][r][cH]
            for qq in range(2):
                row = (2 * pp + qq) * 32
                nc.sync.dma_start(
                    out=d_scr[pp, :, :, qq, :, :],
                    in_=idx16[row: row + 18, :]
                    .rearrange("p (cl r ch) -> p cl r ch", cl=16, r=8, ch=8))
            # wrap: idxw[g*16+cL, k, (qq,r,cH)] broadcast over the 4
            # groups of each deform-group's 64 channel partitions
            # (one DMA per (k,dd): the 3-dim AP balancing limit forbids
            # consolidating the k axis into the broadcast)
            for k in range(9):
                for dd in range(2):
                    src = d_scr[pp, 2 * k + dd: 2 * k + dd + 1] \
                        .rearrange("t cl q r ch -> t cl (q r ch)") \
                        .to_broadcast([4, 16, 128])
                    nc.sync.dma_start(
                        out=idxw_t[pp][dd * 64:(dd + 1) * 64, k, 0:128],
                        in_=src)
            # y1 = y0 + FC for all 9 taps at once
            nc.vector.tensor_scalar(idxw_t[pp][:, :, 128:256],
                                    idxw_t[pp][:, :, 0:128],
                                    FC, None, AL.add)

        def phase_b_cw(pp):
            # corner weights (mask folded in): cw[:, yc, :, xc]
            s = slice(64 * pp, 64 * pp + 64)
            nc.vector.tensor_tensor(fy[s], m_t[s], fy[s], AL.mult)       # u1
            nc.vector.tensor_tensor(m_t[s], m_t[s], fy[s], AL.subtract)  # u0
            nc.vector.tensor_tensor(cw[s, 0, :, 1], m_t[s], fx[s], AL.mult)
            nc.vector.tensor_tensor(cw[s, 0, :, 0], m_t[s],
                                    cw[s, 0, :, 1], AL.subtract)
            nc.vector.tensor_tensor(cw[s, 1, :, 1], fy[s], fx[s], AL.mult)
            nc.vector.tensor_tensor(cw[s, 1, :, 0], fy[s],
                                    cw[s, 1, :, 1], AL.subtract)
            # export corner weights: d_cw[pp][t][yc][qq][r][c][xc]
            for qq in range(2):
                rowp = (2 * pp + qq) * 32
                nc.sync.dma_start(
                    out=d_cw[pp, :, :, qq, :, :, :],
                    in_=cw[rowp: rowp + 18, :, :, :]
                    .rearrange("p y (r c) x -> p y r c x", r=8))

        # ---------------- head: everything pass-0's DCN needs -----------
        for c in range(5):
            conv1_chunk(c)
        conv2_chunk(0)
        conv1_chunk(5)
        conv2_chunk(1)
        conv1_chunk(6)
        conv2_chunk(2)
        off_chunk(0)
        conv2_chunk(3)
        off_chunk(1)
        conv2_chunk(4)
        off_chunk(2)
        conv2_chunk(5)
        off_chunk(3)
        phase_b_idx(0)
        phase_b_cw(0)

        # ------------- pass-0 k-loop with deferred filler work ----------
        # filler[k] runs after tap k's main matmuls: PE work pass-0
        # doesn't depend on, sized to the per-k DMA-bound pipeline gap,
        # plus pass-1's index/weight prep on the DVE.
        def pass_c(pp, filler=()):
            pos = [psum_out.tile([P, NCHUNK], F32, name=f"pos{pp}_{qq}",
                                 tag=f"pos{qq}") for qq in range(2)]
            fdt = fd_a if pp == 0 else fd_b
            for k in range(9):
                # gather x-pairs as single int32 elements
                g = gb_pool.tile([P, 4096, 2], F16)
                nc.gpsimd.ap_gather(
                    g[:].bitcast(I32), fdt[:].bitcast(I32)[:, 0:INW],
                    idxw_t[pp][:, k, :], channels=P,
                    num_elems=INW, d=1, num_idxs=4096)
                g16 = g[:].rearrange("p a b -> p (a b)")
                rtvs = []
                for yc in range(2):
                    # corner-weight partition broadcast: one DMA, 128
                    # descriptors of 8KB (row dd -> partitions dd*64..+64)
                    wcb = wcb_pool.tile([P, 4096], F16)
                    nc.sync.dma_start(
                        out=wcb[:],
                        in_=d_cw[pp, 2 * k: 2 * k + 2, yc]
                        .rearrange("t q r c x -> t (q r c x)")[:, None]
                        .to_broadcast([2, 64, 4096]))
                    # Hadamard: one fp16 TT in 2x DVE mode per y corner
                    rt = rt_pool.tile([P, 4096], F16)
                    nc.vector.tensor_tensor(
                        rt[:], wcb[:], g16[:, yc * 4096:(yc + 1) * 4096],
                        AL.mult)
                    rtvs.append(rt[:].rearrange("p (q r c x) -> p q r c x",
                                                q=2, r=8, x=2))
                # one dense 16-matmul burst per tap (half the cold p-state
                # starts; PE isn't the pacing engine, so starting after
                # both Hadamards is free)
                for yc in range(2):
                    for xc in range(2):
                        for qq in range(2):
                            for h5 in range(2):
                                rhs = rtvs[yc][:, qq,
                                               4 * h5: 4 * h5 + 4, :, xc]
                                nc.tensor.matmul(
                                    pos[qq][:, h5 * 512:(h5 + 1) * 512],
                                    ldcn[:, k, :], rhs,
                                    start=(k == 0 and yc == 0 and xc == 0),
                                    stop=(k == 8 and yc == 1 and xc == 1))
                if k < len(filler):
                    for fn in filler[k]:
                        fn()
            for qq in range(2):
                ob = out_pool.tile([P, NCHUNK], F16, tag="ob")
                nc.scalar.activation(ob[:], pos[qq][:], AF.Relu,
                                     bias=beta3, scale=inv3)
                nc.sync.dma_start(out=d_out[:, pp, qq], in_=ob[:])

        fill0 = [
            [lambda: conv1_chunk(7), lambda: conv2_chunk(6)],
            [lambda: off_chunk(4), lambda: conv1_chunk(8)],
            [lambda: conv2_chunk(7), lambda: off_chunk(5)],
            [lambda: conv1_chunk(9), lambda: conv2_chunk(8)],
            [lambda: off_chunk(6), lambda: conv2_chunk(9)],
            [lambda: off_chunk(7)],
            [lambda: phase_b_idx(1)],
            [lambda: phase_b_cw(1)],
        ]
        pass_c(0, fill0)
        pass_c(1)

    nc.compile()
    return nc


_CACHE = {}


def _prep(inputs):
    f = {k: _f(v) for k, v in inputs.items()}
    inv1 = f['g1'] / np.sqrt(f['v1'] + EPS)
    beta1 = f['b1'] - f['m1'] * inv1
    inv2 = f['g2'] / np.sqrt(f['v2'] + EPS)
    beta2 = f['b2'] - f['m2'] * inv2
    invd = f['gd'] / np.sqrt(f['vd'] + EPS)
    betad = f['bd'] - f['md'] * invd
    inv3 = f['g3'] / np.sqrt(f['v3'] + EPS)
    beta3 = f['b3'] - f['m3'] * inv3

    lhsT1 = np.transpose(f['w1'], (1, 2, 3, 0)).reshape(Ci, 9, P)
    lhsT2 = np.transpose(f['w2'], (1, 2, 3, 0)).reshape(P, 9, P)
    wd = f['wd'][:, :, 0, 0] * (invd / inv2)[:, None]
    lhsT_sc = np.ascontiguousarray(wd.T)

    # offset conv rows: (k*2+d)*3 + quant  <-  orig quant*18 + d*9 + k
    perm = np.zeros(54, dtype=np.int64)
    for quant in range(3):
        for kk in range(9):
            for dd in range(2):
                perm[quant * 18 + kk * 2 + dd] = quant * 18 + dd * 9 + kk
    ow = f['off_w'][perm]
    obias = f['off_b'][perm]
    lhsT_off = np.transpose(ow, (1, 2, 3, 0)).reshape(P, 9, 54)

    wr = f['dcn_w'].reshape(Co, DG, Cg, 9)
    lhsT_dcn = np.transpose(wr, (1, 2, 3, 0)).reshape(P, 9, Co)

    cst = np.zeros((P, 8), dtype=np.float32)
    cst[:, 0], cst[:, 1] = inv1, beta1
    cst[:, 2], cst[:, 3] = inv2, beta2 + betad
    cst[:, 4], cst[:, 5] = inv3, beta3 + inv3 * f['dcn_b']

    bias_q = np.zeros((P, 3), dtype=np.float32)
    for pq in range(4):
        for kk in range(9):
            for dd in range(2):
                r = pq * 32 + kk * 2 + dd
                for quant in range(3):
                    bias_q[r, quant] = obias[quant * 18 + kk * 2 + dd]

    # coordinate base maps (pass-relative row base, offset bias folded in)
    base_y = np.zeros((P, NCHUNK), dtype=np.float16)
    base_x = np.zeros((P, NCHUNK), dtype=np.float16)
    px = np.arange(NCHUNK)
    for p in range(P):
        pq, t = p // 32, p % 32
        if t >= 18:
            continue
        kk, dd = t // 2, t % 2
        ky, kx = kk // 3, kk % 3
        base_y[p] = 8 * (pq % 2) + px // 128 + ky + 2 + bias_q[p, 0]
        base_x[p] = px % 128 + kx + 2 + bias_q[p, 1]

    return dict(
        lhsT1=_h(lhsT1), lhsT2=_h(lhsT2), lhsT_sc=_h(lhsT_sc),
        lhsT_off=_h(lhsT_off), lhsT_dcn=_h(lhsT_dcn),
        consts=_f(cst), bias_q=_f(bias_q), base_y=base_y, base_x=base_x,
        x=f['x'])


def kernel(**inputs):
    cfg = _prep(inputs)
    x = cfg.pop('x')
    B = x.shape[0]

    if 'nc' not in _CACHE:
        _CACHE['nc'] = build_nc()
    nc = _CACHE['nc']

    in_maps = []
    for cid in range(8):
        b, q = cid // 4, cid % 4
        h0 = 32 * q
        xp = np.zeros((Ci, XR, XC), dtype=np.float16)
        r_lo = 2 * h0 - 9
        s_lo, s_hi = max(r_lo, 0), min(2 * h0 + 72, 256)
        xp[:, s_lo - r_lo: s_hi - r_lo, 1:257] = \
            x[b, :, s_lo:s_hi, :].astype(np.float16)
        rm1 = np.zeros((P, F1R), dtype=np.float16)
        for f1 in range(F1R):
            rm1[:, f1] = 1.0 if 0 <= h0 - 4 + f1 < H else 0.0
        rmf = np.zeros((P, FR), dtype=np.float16)
        for f2 in range(FR):
            rmf[:, f2] = 1.0 if 0 <= h0 - 3 + f2 < H else 0.0
        m = dict(cfg)
        m['x_shard'] = np.ascontiguousarray(xp)
        m['rowmask1'] = rm1
        m['rowmaskF'] = rmf
        in_maps.append(m)

    res = run_bass_kernel_spmd(nc, in_maps, core_ids=list(range(8)))
    out = np.zeros((B, Co, H, W), dtype=np.float32)
    for cid in range(8):
        b, q = cid // 4, cid % 4
        o = res.results[cid]['out']            # [P, pp, qq, (r, c)] f16
        out[b, :, 32 * q:32 * q + 32, :] = \
            o.reshape(P, 32, 128).astype(np.float32)
    return out
